# revision 24
# baseline (speedup 1.0000x reference)
"""PhysicsInformedLoss on 8 Trainium2 NeuronCores.

Sharding strategy (degree-class padded CSR):
- Edges are grouped by destination node `row` (the scatter target of every
  segment-mean in the reference). Nodes with deg>0 are binned into degree
  classes K (multiples of 4, small classes merged); each node gets exactly K
  contiguous "slots" (its edges + self-pads, pads contribute exactly 0).
- Nodes of each class are split evenly across the 8 cores (identical padded
  per-core counts -> one SPMD program). Per core, node i of a class maps to
  partition p = i // q (q nodes/partition), so every per-node segment sum is
  a static strided reduction along the free dimension.
- The host gathers the 7 col-side planes (pos xyz, vel uvw, p) in slot order
  (this is the "shard the edges" data layout step); the row side is the
  per-node resident plane broadcast along K by a stride-0 access pattern.
- Device per core: stream col planes, compute all per-edge terms, strided
  per-node reduce, finish div/residual/squares; output per-partition partial
  sums. Host sums 8x128 partials and forms the scalar loss.
"""
import contextlib
import ctypes
import os
import sys
import tempfile
import types

import numpy as np

import concourse.bass as bass
import concourse.tile as tile
from concourse import mybir
from concourse.vector_clock import ScopedClock
from concourse.bass_utils import run_bass_kernel_spmd

N_CORES = 8
P = 128
EPS = 1e-8
REYNOLDS = 1000000.0
LAMBDA_CONT = 0.1
LAMBDA_MOM = 0.01
F_TILE = 1152  # target per-partition columns per tile

# ---------------------------------------------------------------- tile patch
# walrus in this environment allows only ONE sync-wait per instruction, but
# Tile's scheduler can emit several. Split surplus waits onto engine NOPs
# inserted right before the offending instruction.
_MAX_WAITS = 1


def _split_multi_waits(nc, handles):
    work = []
    for fn in nc.m.functions:
        for bb in fn.blocks:
            items = []
            for inst in bb.instructions:
                si = inst.sync_info
                waits = list(si.on_wait) if si and si.on_wait else []
                if len(waits) > _MAX_WAITS:
                    keep = len(waits) - _MAX_WAITS
                    extra = waits[:keep]
                    si.on_wait = waits[keep:]
                    chunks = [
                        extra[i : i + _MAX_WAITS]
                        for i in range(0, len(extra), _MAX_WAITS)
                    ]
                    items.append((inst.name, inst.engine, chunks))
            if items:
                work.append((bb, items))
    if not work:
        return
    created = {}
    placements = {}
    for bb, items in work:
        plc = {}
        for inst_name, engine, chunks in items:
            nops = []
            for chunk in chunks:
                for w in chunk:
                    h = handles.get(w.ant_name)
                    assert h is not None, f"no sem handle for {w.ant_name}"
                    ni = nc.engines[engine].wait_ge(h, w.wait_value)
                    created[ni.ins.name] = None
                    nops.append(ni.ins)
            plc[inst_name] = nops
        placements[id(bb)] = plc
    for fn in nc.m.functions:
        for bb in fn.blocks:
            plc = placements.get(id(bb), {})
            newlist = []
            for inst in bb.instructions:
                if inst.name in created:
                    continue
                if inst.name in plc:
                    newlist.extend(plc[inst.name])
                newlist.append(inst)
            bb.instructions = newlist


def _patched_drain_and_barrier(self, tick_clock, wait_clock):
    drain_inst = self.nc.sync.drain()
    wait_clock.add_sem_waits(
        drain_inst.ins, ScopedClock({None: tick_clock.global_clock})
    )
    handles = {h.name: h for h in self.sems.allocated().values()}
    _split_multi_waits(self.nc, handles)
    self.nc.all_engine_barrier()
    popped = self.nc._tile_sem_poison_stack.pop()
    assert popped is self._sem_poison
    self.nc.clear_and_free_semaphores(list(self.sems.allocated().values()))
    self.nc.all_engine_barrier()


tile.TileContext._drain_and_barrier = _patched_drain_and_barrier

# ------------------------------------------------------------- ntff hook
# The env's antenv package lacks axon_hooks; recreate the NTFF profile hook
# via ctypes so run_bass_kernel_spmd(trace=True) works (test/profiling only).
_AXON_SO = "/opt/axon/libaxon_pjrt.so"


def _install_ntff_hook():
    if "antenv.axon_hooks" in sys.modules:
        return
    try:
        lib = ctypes.CDLL(_AXON_SO)
        lib.axon_start_nrt_profile.argtypes = [
            ctypes.POINTER(ctypes.c_int64),
            ctypes.c_size_t,
        ]
        lib.axon_start_nrt_profile.restype = ctypes.c_int64
        lib.axon_stop_nrt_profile.argtypes = [ctypes.c_char_p]
        lib.axon_stop_nrt_profile.restype = ctypes.c_int64
    except Exception:
        return

    @contextlib.contextmanager
    def _hook(output_dir, device_ids):
        import jax

        jax.devices()
        if device_ids:
            ids = (ctypes.c_int64 * len(device_ids))(*device_ids)
            rc = lib.axon_start_nrt_profile(ids, len(device_ids))
        else:
            rc = lib.axon_start_nrt_profile(None, 0)
        if rc != 0:
            raise RuntimeError(f"axon_start_nrt_profile rc={rc}")
        try:
            yield
        finally:
            n = lib.axon_stop_nrt_profile(str(output_dir).encode())
            print(f"profile: {n} file(s) written to {output_dir}", file=sys.stderr)

    mod = types.ModuleType("antenv.axon_hooks")
    mod.get_axon_ntff_profile_hook = lambda: _hook
    mod.set_axon_ntff_profile_hook = lambda h: None
    sys.modules["antenv.axon_hooks"] = mod


# ---------------------------------------------------------------- host prep


def _build_plan(row, n, min_class_slots=128 * 1024):
    deg = np.bincount(row, minlength=n).astype(np.int64)
    order = np.argsort(row, kind="stable")
    offs = np.zeros(n + 1, dtype=np.int64)
    np.cumsum(deg, out=offs[1:])

    kraw = ((np.maximum(deg, 1) + 3) // 4) * 4
    active = deg > 0
    uniq = np.unique(kraw[active])
    classes = []
    pend = []
    pend_slots = 0
    for K in uniq:
        ids = np.nonzero(active & (kraw == K))[0]
        pend.append(ids)
        pend_slots += ids.size * int(K)
        if pend_slots >= min_class_slots or K == uniq[-1]:
            allids = np.concatenate(pend)
            classes.append((int(K), allids))
            pend = []
            pend_slots = 0
    plan = []
    for K, ids in classes:
        m = -(-ids.size // (N_CORES * P)) * P
        plan.append((K, ids, m))
    return plan, deg, offs, order


def _build_streams(plan, deg, offs, col_sorted, nodedata):
    """nodedata: [n,7] f32. Returns (cores list of dicts, NN, S)."""
    import ml_dtypes

    bf = ml_dtypes.bfloat16
    S = sum(m * K for K, _, m in plan)
    NN = sum(m for _, _, m in plan)
    cores = []
    for c in range(N_CORES):
        col_pos = np.zeros((3, P, S // P), np.float32)
        col_vel = np.zeros((4, P, S // P), bf)
        node_pos = np.zeros((3, P, NN // P), np.float32)
        node_vel = np.zeros((4, P, NN // P), bf)
        cnt = np.ones((P, NN // P), np.float32)
        off_slots = 0
        off_nodes = 0
        for K, ids, m in plan:
            q = m // P
            ids_c = ids[c * m : (c + 1) * m]
            k_real = ids_c.size
            vals = np.zeros((m, K, 7), np.float32)
            if k_real > 0:
                colmat = np.empty((k_real, K), np.int64)
                colmat[:] = ids_c[:, None]
                dd = deg[ids_c]
                oo = offs[ids_c]
                ar = np.arange(K)[None, :]
                valid = ar < dd[:, None]
                src_idx = (oo[:, None] + ar)[valid]
                colmat[valid] = col_sorted[src_idx]
                vals[:k_real] = nodedata[colmat]
                nodevals = np.zeros((m, 7), np.float32)
                nodevals[:k_real] = nodedata[ids_c]
                cv = np.ones(m, np.float32)
                cv[:k_real] = np.maximum(dd, 1).astype(np.float32)
            else:
                nodevals = np.zeros((m, 7), np.float32)
                cv = np.ones(m, np.float32)
            w = vals.reshape(P, q, K, 7).transpose(3, 0, 1, 2).reshape(7, P, q * K)
            col_pos[:, :, off_slots : off_slots + q * K] = w[:3]
            col_vel[:, :, off_slots : off_slots + q * K] = w[3:].astype(bf)
            nv = nodevals.reshape(P, q, 7).transpose(2, 0, 1)
            node_pos[:, :, off_nodes : off_nodes + q] = nv[:3]
            node_vel[:, :, off_nodes : off_nodes + q] = nv[3:].astype(bf)
            cnt[:, off_nodes : off_nodes + q] = cv.reshape(P, q)
            off_slots += q * K
            off_nodes += q
        cores.append(
            dict(
                col_pos=col_pos, col_vel=col_vel, node_pos=node_pos,
                node_vel=node_vel, cnt=cnt,
            )
        )
    return cores, NN, S


# ---------------------------------------------------------------- bass build


def _class_tiles(plan):
    """Yield (K, q_nodes_in_tile, slot_col_offset, node_col_offset) splits."""
    tiles = []
    off_s = 0
    off_n = 0
    for K, _, m in plan:
        q = m // P
        # split q nodes into groups of ~F_TILE/K
        gmax = max(1, F_TILE // K)
        i = 0
        while i < q:
            g = min(gmax, q - i)
            tiles.append((K, g, off_s + i * K, off_n + i))
            i += g
        off_s += q * K
        off_n += q
    return tiles


def _raw_scalar_act(nc, out, in_, func, bias=0.0, scale=1.0):
    """InstActivation without the python wrapper's Reciprocal ban and without
    the const-AP bias conversion (immediates work for these funcs here)."""
    inputs = [nc.scalar.lower_ap(in_)]
    for arg in (bias, scale, 0.0):
        inputs.append(mybir.ImmediateValue(dtype=mybir.dt.float32, value=arg))
    return nc.scalar.add_instruction(
        mybir.InstActivation(
            name=nc.get_next_instruction_name(),
            func=func,
            ins=inputs,
            outs=[nc.scalar.lower_ap(out)],
        )
    )


def _build_nc(plan, NN, S, DQ):
    """Build the SPMD bass program. DQ = per-partition cols of data-loss
    planes (4 planes each for pred/target slices)."""
    fp32 = mybir.dt.float32
    bf16 = mybir.dt.bfloat16
    nc = bass.Bass("TRN2", target_bir_lowering=False)
    W = S // P
    Q = NN // P

    cpos = [
        nc.dram_tensor(f"cpos{i}", [P, W], fp32, kind="ExternalInput")
        for i in range(3)
    ]
    cvel = [
        nc.dram_tensor(f"cvel{i}", [P, W], bf16, kind="ExternalInput")
        for i in range(4)
    ]
    nodp = nc.dram_tensor("nodp", [P, 3 * Q], fp32, kind="ExternalInput")
    nodv = nc.dram_tensor("nodv", [P, 4 * Q], bf16, kind="ExternalInput")
    cntT = nc.dram_tensor("cnt", [P, Q], fp32, kind="ExternalInput")
    dlp = nc.dram_tensor("dlp", [P, 4 * DQ], fp32, kind="ExternalInput")
    dlt = nc.dram_tensor("dlt", [P, 4 * DQ], fp32, kind="ExternalInput")
    out = nc.dram_tensor("out", [P, 8], fp32, kind="ExternalOutput")

    AF = mybir.ActivationFunctionType
    OP = mybir.AluOpType

    with tile.TileContext(nc) as tc:
        with (
            tc.tile_pool(name="resident", bufs=1) as res_pool,
            tc.tile_pool(name="colp", bufs=2) as col_pool,
            tc.tile_pool(name="exp", bufs=2) as exp_pool,
            tc.tile_pool(name="tmp", bufs=2) as tmp_pool,
            tc.tile_pool(name="tmp1", bufs=1) as tmp1_pool,
        ):
            # resident: node planes, cnt, accumulators
            nodpt = res_pool.tile([P, 3 * Q], fp32)
            nc.sync.dma_start(nodpt[:], nodp.ap()[:])
            nodvt = res_pool.tile([P, 4 * Q], bf16)
            nc.sync.dma_start(nodvt[:], nodv.ap()[:])
            cntt = res_pool.tile([P, Q], fp32)
            nc.sync.dma_start(cntt[:], cntT.ap()[:])
            acc = res_pool.tile([P, 7 * Q], fp32)  # g, lx, ly, lz, px, py, pz

            # ---- data loss (small, fp32) ----
            dlpt = res_pool.tile([P, 4 * DQ], fp32)
            nc.sync.dma_start(dlpt[:], dlp.ap()[:])
            dltt = res_pool.tile([P, 4 * DQ], fp32)
            nc.sync.dma_start(dltt[:], dlt.ap()[:])
            dld = res_pool.tile([P, 4 * DQ], fp32)
            nc.vector.tensor_sub(dld[:], dlpt[:], dltt[:])
            acc_vel = res_pool.tile([P, 1], fp32)
            acc_pres = res_pool.tile([P, 1], fp32)
            nc.scalar.activation(
                dld[:, 0 : 3 * DQ], dld[:, 0 : 3 * DQ], AF.Square,
                accum_out=acc_vel[:],
            )
            nc.scalar.activation(
                dld[:, 3 * DQ : 4 * DQ], dld[:, 3 * DQ : 4 * DQ], AF.Square,
                accum_out=acc_pres[:],
            )

            # ---- main loop ----
            # vals plane order: [velgrad, lu, lv, lw, pgx, pgy, pgz]
            acc4 = acc[:].rearrange("p (i q) -> p i q", i=7, q=Q)
            for K, g, off_s, off_n in _class_tiles(plan):
                F = g * K
                cpt = col_pool.tile([P, 3 * F], fp32, tag="cpt", name="cpt")
                cvt = col_pool.tile([P, 4 * F], bf16, tag="cvt", name="cvt")
                for i in range(3):
                    nc.sync.dma_start(
                        cpt[:, i * F : (i + 1) * F],
                        cpos[i].ap()[:, off_s : off_s + F],
                    )
                for i in range(4):
                    nc.sync.dma_start(
                        cvt[:, i * F : (i + 1) * F],
                        cvel[i].ap()[:, off_s : off_s + F],
                    )

                def bcast4(plane, nplanes, qq):
                    # [P, nplanes, g, K] broadcast of node planes
                    a = plane[:].rearrange("p (i q) -> p i q", i=nplanes, q=qq)
                    return a[:, :, off_n : off_n + g].unsqueeze(-1).broadcast_to(
                        [P, nplanes, g, K]
                    )

                def pv(t, n, lo, hi):
                    # planes [lo:hi) of an n-plane tile as [P, hi-lo, g, K]
                    return t[:].rearrange("p (i g k) -> p i g k", i=n, g=g, k=K)[
                        :, lo:hi
                    ]

                # node vel/p expand materialized on ScalarE (one op, 4 planes)
                evt = exp_pool.tile([P, 4 * F], bf16, tag="evt", name="evt")
                nc.scalar.copy(pv(evt, 4, 0, 4), bcast4(nodvt, 4, Q))

                # position diffs fp32 (one op, broadcast in1, 1x)
                dxt = tmp1_pool.tile([P, 3 * F], fp32, tag="dxt", name="dxt")
                nc.vector.tensor_sub(pv(dxt, 3, 0, 3), pv(cpt, 3, 0, 3),
                                     bcast4(nodpt, 3, Q))

                # squares (ScalarE, one op); r2 = sum of the 3 square planes
                # via SWDGE CCE-accumulate DMAs (keeps DVE free)
                sqt = tmp_pool.tile([P, 3 * F], fp32, tag="sqt", name="sqt")
                nc.scalar.activation(sqt[:], dxt[:], AF.Square)
                nc.gpsimd.dma_start(
                    sqt[:, 0:F], sqt[:, F : 2 * F], accum_op=OP.add
                )
                nc.gpsimd.dma_start(
                    sqt[:, 0:F], sqt[:, 2 * F : 3 * F], accum_op=OP.add
                )
                r2 = sqt[:, 0:F]

                # d1 = 1/(sqrt(r2)+eps), d2 = 1/(r2+eps): ScalarE LUTs -> bf16
                ss = tmp_pool.tile([P, F], fp32, tag="ss", name="ss")
                nc.scalar.activation(ss[:], r2, AF.Sqrt)
                d1 = tmp_pool.tile([P, F], bf16, tag="d1")
                _raw_scalar_act(nc, d1[:], ss[:], AF.Reciprocal, bias=EPS)
                d2 = tmp_pool.tile([P, F], bf16, tag="d2")
                _raw_scalar_act(nc, d2[:], r2, AF.Reciprocal, bias=EPS)

                vals = tmp1_pool.tile([P, 7 * F], bf16, tag="vals", name="vals")
                # dxb,dyb,dzb -> vals planes 4:7 (one CAST)
                nc.vector.tensor_copy(vals[:, 4 * F : 7 * F], dxt[:])
                # du,dv,dw -> vals planes 1:4 ; dq separate
                nc.vector.tensor_sub(
                    vals[:, 1 * F : 4 * F], cvt[:, 0 : 3 * F], evt[:, 0 : 3 * F]
                )
                dqt = tmp1_pool.tile([P, F], bf16, tag="dqt", name="dqt")
                nc.vector.tensor_sub(dqt[:], cvt[:, 3 * F :], evt[:, 3 * F :])

                # g = sum duvw*dxyz ; velgrad = g*d1 -> vals plane 0
                prt = tmp1_pool.tile([P, 3 * F], bf16, tag="prt", name="prt")
                nc.vector.tensor_mul(
                    prt[:], vals[:, 1 * F : 4 * F], vals[:, 4 * F : 7 * F]
                )
                nc.gpsimd.dma_start(
                    prt[:, 0:F], prt[:, F : 2 * F], accum_op=OP.add
                )
                nc.gpsimd.dma_start(
                    prt[:, 0:F], prt[:, 2 * F : 3 * F], accum_op=OP.add
                )
                nc.vector.tensor_mul(vals[:, 0:F], prt[:, 0:F], d1[:])

                # cp = dq*d1*d1 (in-place dqt)
                nc.vector.tensor_mul(dqt[:], dqt[:], d1[:])
                nc.vector.tensor_mul(dqt[:], dqt[:], d1[:])

                # pg = cp * dxyz (planes 4:7, in-place; cp broadcast over 3)
                cp3 = bass.AP(dqt[:].tensor, dqt[:].offset,
                              [dqt[:].ap[0], [0, 3], [1, F]])
                nc.vector.tensor_mul(
                    vals[:].rearrange("p (i f) -> p i f", i=7, f=F)[:, 4:7],
                    cp3,
                    vals[:].rearrange("p (i f) -> p i f", i=7, f=F)[:, 4:7],
                )
                # lap = duvw * d2 (planes 1:4, in-place; d2 broadcast over 3)
                d23 = bass.AP(d2[:].tensor, d2[:].offset,
                              [d2[:].ap[0], [0, 3], [1, F]])
                nc.vector.tensor_mul(
                    vals[:].rearrange("p (i f) -> p i f", i=7, f=F)[:, 1:4],
                    vals[:].rearrange("p (i f) -> p i f", i=7, f=F)[:, 1:4],
                    d23,
                )

                # segment sums: halving tree over K (all 7 planes at once)
                v4 = vals[:].rearrange("p (i g k) -> p i g k", i=7, g=g, k=K)
                k = K
                h = k // 2
                nc.vector.tensor_add(
                    v4[:, :, :, 0:h], v4[:, :, :, 0:h], v4[:, :, :, h:k]
                )
                k = h
                if k % 2 == 0 and (k // 2) % 2 == 0:
                    h = k // 2
                    nc.vector.tensor_add(
                        v4[:, :, :, 0:h], v4[:, :, :, 0:h], v4[:, :, :, h:k]
                    )
                    k = h
                nc.vector.tensor_reduce(
                    acc4[:, :, off_n : off_n + g],
                    v4[:, :, :, 0:k],
                    mybir.AxisListType.X,
                    OP.add,
                )

            # ---- finish (fp32, small) ----
            icnt = res_pool.tile([P, Q], fp32)
            _raw_scalar_act(nc, icnt[:], cntt[:], AF.Reciprocal)
            div = res_pool.tile([P, Q], fp32)
            nc.vector.tensor_mul(div[:], acc[:, 0:Q], icnt[:])
            acc_div2 = res_pool.tile([P, 1], fp32)
            nc.scalar.activation(div[:], div[:], AF.Square, accum_out=acc_div2[:])
            acc_m = [
                res_pool.tile([P, 1], fp32, tag=f"am{i}", name=f"am{i}")
                for i in range(3)
            ]
            for i in range(3):
                r = res_pool.tile([P, Q], fp32, tag="rfin")
                nc.vector.scalar_tensor_tensor(
                    r[:],
                    acc[:, (1 + i) * Q : (2 + i) * Q],
                    1.0 / REYNOLDS,
                    acc[:, (4 + i) * Q : (5 + i) * Q],
                    OP.mult,
                    OP.add,
                )
                nc.vector.tensor_mul(r[:], r[:], icnt[:])
                nc.scalar.activation(r[:], r[:], AF.Square, accum_out=acc_m[i][:])

            outt = res_pool.tile([P, 8], fp32)
            nc.vector.memset(outt[:], 0.0)
            nc.vector.tensor_copy(outt[:, 0:1], acc_vel[:])
            nc.vector.tensor_copy(outt[:, 1:2], acc_pres[:])
            nc.vector.tensor_copy(outt[:, 2:3], acc_div2[:])
            nc.vector.tensor_copy(outt[:, 3:4], acc_m[0][:])
            nc.vector.tensor_copy(outt[:, 4:5], acc_m[1][:])
            nc.vector.tensor_copy(outt[:, 5:6], acc_m[2][:])
            nc.sync.dma_start(out.ap()[:], outt[:])

    return nc


# ---------------------------------------------------------------- entry

_CACHE = {}


def _get_nc(key, plan, NN, S, DQ):
    if key not in _CACHE:
        _CACHE[key] = _build_nc(plan, NN, S, DQ)
    return _CACHE[key]


LAST_RESULT = None  # BassKernelResults of the most recent run (for profiling)


def kernel(pred, target, edge_index, pos, _trace_dir=None):
    global LAST_RESULT
    pred = np.asarray(pred)
    target = np.asarray(target)
    pos = np.asarray(pos)
    row = np.asarray(edge_index[0]).astype(np.int64)
    col = np.asarray(edge_index[1]).astype(np.int64)
    n = pred.shape[0]

    plan, deg, offs, order = _build_plan(row, n)
    col_sorted = col[order]
    nodedata = np.concatenate(
        [pos.astype(np.float32), pred.astype(np.float32)], axis=1
    )
    cores, NN, S = _build_streams(plan, deg, offs, col_sorted, nodedata)

    # data-loss slices: split all n nodes across cores, pad to mult of 128
    per = -(-n // N_CORES)
    DQ = (-(-per // P) * P) // P
    predf = pred.astype(np.float32)
    targf = target.astype(np.float32)

    in_maps = []
    for c in range(N_CORES):
        lo, hi = c * per, min((c + 1) * per, n)
        ps = np.zeros((P * DQ, 4), np.float32)
        ts = np.zeros((P * DQ, 4), np.float32)
        ps[: hi - lo] = predf[lo:hi]
        ts[: hi - lo] = targf[lo:hi]
        # [P, 4*DQ] with plane-major columns: plane i at cols [i*DQ, (i+1)*DQ)
        dlp = ps.reshape(P, DQ, 4).transpose(0, 2, 1).reshape(P, 4 * DQ)
        dlt = ts.reshape(P, DQ, 4).transpose(0, 2, 1).reshape(P, 4 * DQ)
        Qn = NN // P
        m = dict(
            cnt=np.ascontiguousarray(cores[c]["cnt"]),
            nodp=np.ascontiguousarray(
                cores[c]["node_pos"].transpose(1, 0, 2).reshape(P, 3 * Qn)
            ),
            nodv=np.ascontiguousarray(
                cores[c]["node_vel"].transpose(1, 0, 2).reshape(P, 4 * Qn)
            ),
            dlp=np.ascontiguousarray(dlp),
            dlt=np.ascontiguousarray(dlt),
        )
        for i in range(3):
            m[f"cpos{i}"] = np.ascontiguousarray(cores[c]["col_pos"][i])
        for i in range(4):
            m[f"cvel{i}"] = np.ascontiguousarray(cores[c]["col_vel"][i])
        in_maps.append(m)

    key = (tuple((K, m) for K, _, m in plan), NN, S, DQ)
    nc = _get_nc(key, plan, NN, S, DQ)

    if _trace_dir is not None:
        _install_ntff_hook()
        res = run_bass_kernel_spmd(
            nc, in_maps, core_ids=list(range(N_CORES)), trace=True,
            tmpdir=_trace_dir,
        )
    else:
        res = run_bass_kernel_spmd(nc, in_maps, core_ids=list(range(N_CORES)))
    LAST_RESULT = res

    tot = np.zeros(8, np.float64)
    for c in range(N_CORES):
        tot += res.results[c]["out"].astype(np.float64).sum(axis=0)
    s_vel, s_pres, s_div2, am0, am1, am2 = tot[0], tot[1], tot[2], tot[3], tot[4], tot[5]
    loss = (
        s_vel / (3 * n)
        + s_pres / n
        + LAMBDA_CONT * s_div2 / n
        + LAMBDA_MOM * (am0 + am1 + am2) / (3 * n)
    )
    return np.float32(loss)


# revision 25
# speedup vs baseline: 1.2644x; 1.2644x over previous
"""PhysicsInformedLoss on 8 Trainium2 NeuronCores.

Sharding strategy (degree-class padded CSR):
- Edges are grouped by destination node `row` (the scatter target of every
  segment-mean in the reference). Nodes with deg>0 are binned into degree
  classes K (multiples of 4, small classes merged); each node gets exactly K
  contiguous "slots" (its edges + self-pads, pads contribute exactly 0).
- Nodes of each class are split evenly across the 8 cores (identical padded
  per-core counts -> one SPMD program). Per core, node i of a class maps to
  partition p = i // q (q nodes/partition), so every per-node segment sum is
  a static strided reduction along the free dimension.
- The host gathers the 7 col-side planes (pos xyz, vel uvw, p) in slot order
  (this is the "shard the edges" data layout step); the row side is the
  per-node resident plane broadcast along K by a stride-0 access pattern.
- Device per core: stream col planes, compute all per-edge terms, strided
  per-node reduce, finish div/residual/squares; output per-partition partial
  sums. Host sums 8x128 partials and forms the scalar loss.
"""
import contextlib
import ctypes
import os
import sys
import tempfile
import types

import numpy as np

import concourse.bass as bass
import concourse.tile as tile
from concourse import mybir
from concourse.vector_clock import ScopedClock
from concourse.bass_utils import run_bass_kernel_spmd

N_CORES = 8
P = 128
EPS = 1e-8
REYNOLDS = 1000000.0
LAMBDA_CONT = 0.1
LAMBDA_MOM = 0.01
F_TILE = 1152  # target per-partition columns per tile

# ---------------------------------------------------------------- tile patch
# walrus in this environment allows only ONE sync-wait per instruction, but
# Tile's scheduler can emit several. Split surplus waits onto engine NOPs
# inserted right before the offending instruction.
_MAX_WAITS = 1


def _split_multi_waits(nc, handles):
    work = []
    for fn in nc.m.functions:
        for bb in fn.blocks:
            items = []
            for inst in bb.instructions:
                si = inst.sync_info
                waits = list(si.on_wait) if si and si.on_wait else []
                if len(waits) > _MAX_WAITS:
                    keep = len(waits) - _MAX_WAITS
                    extra = waits[:keep]
                    si.on_wait = waits[keep:]
                    chunks = [
                        extra[i : i + _MAX_WAITS]
                        for i in range(0, len(extra), _MAX_WAITS)
                    ]
                    items.append((inst.name, inst.engine, chunks))
            if items:
                work.append((bb, items))
    if not work:
        return
    created = {}
    placements = {}
    for bb, items in work:
        plc = {}
        for inst_name, engine, chunks in items:
            nops = []
            for chunk in chunks:
                for w in chunk:
                    h = handles.get(w.ant_name)
                    assert h is not None, f"no sem handle for {w.ant_name}"
                    ni = nc.engines[engine].wait_ge(h, w.wait_value)
                    created[ni.ins.name] = None
                    nops.append(ni.ins)
            plc[inst_name] = nops
        placements[id(bb)] = plc
    for fn in nc.m.functions:
        for bb in fn.blocks:
            plc = placements.get(id(bb), {})
            newlist = []
            for inst in bb.instructions:
                if inst.name in created:
                    continue
                if inst.name in plc:
                    newlist.extend(plc[inst.name])
                newlist.append(inst)
            bb.instructions = newlist


def _patched_drain_and_barrier(self, tick_clock, wait_clock):
    drain_inst = self.nc.sync.drain()
    wait_clock.add_sem_waits(
        drain_inst.ins, ScopedClock({None: tick_clock.global_clock})
    )
    handles = {h.name: h for h in self.sems.allocated().values()}
    _split_multi_waits(self.nc, handles)
    self.nc.all_engine_barrier()
    popped = self.nc._tile_sem_poison_stack.pop()
    assert popped is self._sem_poison
    self.nc.clear_and_free_semaphores(list(self.sems.allocated().values()))
    self.nc.all_engine_barrier()


tile.TileContext._drain_and_barrier = _patched_drain_and_barrier

# ------------------------------------------------------------- ntff hook
# The env's antenv package lacks axon_hooks; recreate the NTFF profile hook
# via ctypes so run_bass_kernel_spmd(trace=True) works (test/profiling only).
_AXON_SO = "/opt/axon/libaxon_pjrt.so"


def _install_ntff_hook():
    if "antenv.axon_hooks" in sys.modules:
        return
    try:
        lib = ctypes.CDLL(_AXON_SO)
        lib.axon_start_nrt_profile.argtypes = [
            ctypes.POINTER(ctypes.c_int64),
            ctypes.c_size_t,
        ]
        lib.axon_start_nrt_profile.restype = ctypes.c_int64
        lib.axon_stop_nrt_profile.argtypes = [ctypes.c_char_p]
        lib.axon_stop_nrt_profile.restype = ctypes.c_int64
    except Exception:
        return

    @contextlib.contextmanager
    def _hook(output_dir, device_ids):
        import jax

        jax.devices()
        if device_ids:
            ids = (ctypes.c_int64 * len(device_ids))(*device_ids)
            rc = lib.axon_start_nrt_profile(ids, len(device_ids))
        else:
            rc = lib.axon_start_nrt_profile(None, 0)
        if rc != 0:
            raise RuntimeError(f"axon_start_nrt_profile rc={rc}")
        try:
            yield
        finally:
            n = lib.axon_stop_nrt_profile(str(output_dir).encode())
            print(f"profile: {n} file(s) written to {output_dir}", file=sys.stderr)

    mod = types.ModuleType("antenv.axon_hooks")
    mod.get_axon_ntff_profile_hook = lambda: _hook
    mod.set_axon_ntff_profile_hook = lambda h: None
    sys.modules["antenv.axon_hooks"] = mod


# ---------------------------------------------------------------- host prep


def _build_plan(row, n, min_class_slots=128 * 1024):
    deg = np.bincount(row, minlength=n).astype(np.int64)
    order = np.argsort(row, kind="stable")
    offs = np.zeros(n + 1, dtype=np.int64)
    np.cumsum(deg, out=offs[1:])

    kraw = ((np.maximum(deg, 1) + 3) // 4) * 4
    active = deg > 0
    uniq = np.unique(kraw[active])
    classes = []
    pend = []
    pend_slots = 0
    for K in uniq:
        ids = np.nonzero(active & (kraw == K))[0]
        pend.append(ids)
        pend_slots += ids.size * int(K)
        if pend_slots >= min_class_slots or K == uniq[-1]:
            allids = np.concatenate(pend)
            classes.append((int(K), allids))
            pend = []
            pend_slots = 0
    plan = []
    for K, ids in classes:
        m = -(-ids.size // (N_CORES * P)) * P
        plan.append((K, ids, m))
    return plan, deg, offs, order


def _build_streams(plan, deg, offs, col_sorted, nodedata):
    """nodedata: [n,7] f32. Returns (cores list of dicts, NN, S)."""
    import ml_dtypes

    bf = ml_dtypes.bfloat16
    S = sum(m * K for K, _, m in plan)
    NN = sum(m for _, _, m in plan)
    cores = []
    for c in range(N_CORES):
        col_pos = np.zeros((3, P, S // P), np.float32)
        col_vel = np.zeros((4, P, S // P), bf)
        node_pos = np.zeros((3, P, NN // P), np.float32)
        node_vel = np.zeros((4, P, NN // P), bf)
        cnt = np.ones((P, NN // P), np.float32)
        off_slots = 0
        off_nodes = 0
        for K, ids, m in plan:
            q = m // P
            ids_c = ids[c * m : (c + 1) * m]
            k_real = ids_c.size
            vals = np.zeros((m, K, 7), np.float32)
            if k_real > 0:
                colmat = np.empty((k_real, K), np.int64)
                colmat[:] = ids_c[:, None]
                dd = deg[ids_c]
                oo = offs[ids_c]
                ar = np.arange(K)[None, :]
                valid = ar < dd[:, None]
                src_idx = (oo[:, None] + ar)[valid]
                colmat[valid] = col_sorted[src_idx]
                vals[:k_real] = nodedata[colmat]
                nodevals = np.zeros((m, 7), np.float32)
                nodevals[:k_real] = nodedata[ids_c]
                cv = np.ones(m, np.float32)
                cv[:k_real] = np.maximum(dd, 1).astype(np.float32)
            else:
                nodevals = np.zeros((m, 7), np.float32)
                cv = np.ones(m, np.float32)
            w = vals.reshape(P, q, K, 7).transpose(3, 0, 1, 2).reshape(7, P, q * K)
            col_pos[:, :, off_slots : off_slots + q * K] = w[:3]
            col_vel[:, :, off_slots : off_slots + q * K] = w[3:].astype(bf)
            nv = nodevals.reshape(P, q, 7).transpose(2, 0, 1)
            node_pos[:, :, off_nodes : off_nodes + q] = nv[:3]
            node_vel[:, :, off_nodes : off_nodes + q] = nv[3:].astype(bf)
            cnt[:, off_nodes : off_nodes + q] = cv.reshape(P, q)
            off_slots += q * K
            off_nodes += q
        cores.append(
            dict(
                col_pos=col_pos, col_vel=col_vel, node_pos=node_pos,
                node_vel=node_vel, cnt=cnt,
            )
        )
    return cores, NN, S


# ---------------------------------------------------------------- bass build


def _class_tiles(plan):
    """Yield (K, q_nodes_in_tile, slot_col_offset, node_col_offset) splits."""
    tiles = []
    off_s = 0
    off_n = 0
    for K, _, m in plan:
        q = m // P
        # split q nodes into groups of ~F_TILE/K
        gmax = max(1, F_TILE // K)
        i = 0
        while i < q:
            g = min(gmax, q - i)
            tiles.append((K, g, off_s + i * K, off_n + i))
            i += g
        off_s += q * K
        off_n += q
    return tiles


def _raw_scalar_act(nc, out, in_, func, bias=0.0, scale=1.0):
    """InstActivation without the python wrapper's Reciprocal ban and without
    the const-AP bias conversion (immediates work for these funcs here)."""
    inputs = [nc.scalar.lower_ap(in_)]
    for arg in (bias, scale, 0.0):
        inputs.append(mybir.ImmediateValue(dtype=mybir.dt.float32, value=arg))
    return nc.scalar.add_instruction(
        mybir.InstActivation(
            name=nc.get_next_instruction_name(),
            func=func,
            ins=inputs,
            outs=[nc.scalar.lower_ap(out)],
        )
    )


def _build_nc(plan, NN, S, DQ):
    """Build the SPMD bass program. DQ = per-partition cols of data-loss
    planes (4 planes each for pred/target slices)."""
    fp32 = mybir.dt.float32
    bf16 = mybir.dt.bfloat16
    nc = bass.Bass("TRN2", target_bir_lowering=False)
    W = S // P
    Q = NN // P

    cpos = [
        nc.dram_tensor(f"cpos{i}", [P, W], fp32, kind="ExternalInput")
        for i in range(3)
    ]
    cvel = [
        nc.dram_tensor(f"cvel{i}", [P, W], bf16, kind="ExternalInput")
        for i in range(4)
    ]
    nodp = nc.dram_tensor("nodp", [P, 3 * Q], fp32, kind="ExternalInput")
    nodv = nc.dram_tensor("nodv", [P, 4 * Q], bf16, kind="ExternalInput")
    cntT = nc.dram_tensor("cnt", [P, Q], fp32, kind="ExternalInput")
    dlp = nc.dram_tensor("dlp", [P, 4 * DQ], fp32, kind="ExternalInput")
    dlt = nc.dram_tensor("dlt", [P, 4 * DQ], fp32, kind="ExternalInput")
    out = nc.dram_tensor("out", [P, 8], fp32, kind="ExternalOutput")

    AF = mybir.ActivationFunctionType
    OP = mybir.AluOpType

    with tile.TileContext(nc) as tc:
        with (
            tc.tile_pool(name="resident", bufs=1) as res_pool,
            tc.tile_pool(name="colp", bufs=2) as col_pool,
            tc.tile_pool(name="exp", bufs=2) as exp_pool,
            tc.tile_pool(name="tmp", bufs=2) as tmp_pool,
            tc.tile_pool(name="tmp1", bufs=1) as tmp1_pool,
        ):
            # resident: node planes, cnt, accumulators
            nodpt = res_pool.tile([P, 3 * Q], fp32)
            nc.sync.dma_start(nodpt[:], nodp.ap()[:])
            nodvt = res_pool.tile([P, 4 * Q], bf16)
            nc.sync.dma_start(nodvt[:], nodv.ap()[:])
            cntt = res_pool.tile([P, Q], fp32)
            nc.sync.dma_start(cntt[:], cntT.ap()[:])
            acc = res_pool.tile([P, 7 * Q], fp32)  # g, lx, ly, lz, px, py, pz

            # ---- data loss (small, fp32) ----
            dlpt = res_pool.tile([P, 4 * DQ], fp32)
            nc.sync.dma_start(dlpt[:], dlp.ap()[:])
            dltt = res_pool.tile([P, 4 * DQ], fp32)
            nc.sync.dma_start(dltt[:], dlt.ap()[:])
            dld = res_pool.tile([P, 4 * DQ], fp32)
            nc.vector.tensor_sub(dld[:], dlpt[:], dltt[:])
            acc_vel = res_pool.tile([P, 1], fp32)
            acc_pres = res_pool.tile([P, 1], fp32)
            nc.scalar.activation(
                dld[:, 0 : 3 * DQ], dld[:, 0 : 3 * DQ], AF.Square,
                accum_out=acc_vel[:],
            )
            nc.scalar.activation(
                dld[:, 3 * DQ : 4 * DQ], dld[:, 3 * DQ : 4 * DQ], AF.Square,
                accum_out=acc_pres[:],
            )

            # ---- main loop ----
            # vals plane order: [velgrad, lu, lv, lw, pgx, pgy, pgz]
            acc4 = acc[:].rearrange("p (i q) -> p i q", i=7, q=Q)
            for K, g, off_s, off_n in _class_tiles(plan):
                F = g * K
                cpt = col_pool.tile([P, 3 * F], fp32, tag="cpt", name="cpt")
                cvt = col_pool.tile([P, 4 * F], bf16, tag="cvt", name="cvt")
                for i in range(3):
                    nc.sync.dma_start(
                        cpt[:, i * F : (i + 1) * F],
                        cpos[i].ap()[:, off_s : off_s + F],
                    )
                for i in range(4):
                    nc.sync.dma_start(
                        cvt[:, i * F : (i + 1) * F],
                        cvel[i].ap()[:, off_s : off_s + F],
                    )

                def bcast4(plane, nplanes, qq):
                    # [P, nplanes, g, K] broadcast of node planes
                    a = plane[:].rearrange("p (i q) -> p i q", i=nplanes, q=qq)
                    return a[:, :, off_n : off_n + g].unsqueeze(-1).broadcast_to(
                        [P, nplanes, g, K]
                    )

                def pv(t, n, lo, hi):
                    # planes [lo:hi) of an n-plane tile as [P, hi-lo, g, K]
                    return t[:].rearrange("p (i g k) -> p i g k", i=n, g=g, k=K)[
                        :, lo:hi
                    ]

                # node vel/p expand materialized on ScalarE (one op, 4 planes)
                evt = exp_pool.tile([P, 4 * F], bf16, tag="evt", name="evt")
                nc.scalar.copy(pv(evt, 4, 0, 4), bcast4(nodvt, 4, Q))

                # position diffs fp32 (one op, broadcast in1, 1x)
                dxt = tmp1_pool.tile([P, 3 * F], fp32, tag="dxt", name="dxt")
                nc.vector.tensor_sub(pv(dxt, 3, 0, 3), pv(cpt, 3, 0, 3),
                                     bcast4(nodpt, 3, Q))

                # squares (ScalarE, one op); r2 = sum of the 3 square planes
                # via SWDGE CCE-accumulate DMAs (keeps DVE free)
                sqt = tmp_pool.tile([P, 3 * F], fp32, tag="sqt", name="sqt")
                nc.scalar.activation(sqt[:], dxt[:], AF.Square)
                nc.vector.tensor_add(sqt[:, 0:F], sqt[:, 0:F], sqt[:, F : 2 * F])
                nc.vector.tensor_add(sqt[:, 0:F], sqt[:, 0:F], sqt[:, 2 * F : 3 * F])
                r2 = sqt[:, 0:F]

                # d1 = 1/(sqrt(r2)+eps), d2 = 1/(r2+eps): ScalarE LUTs -> bf16
                ss = tmp_pool.tile([P, F], fp32, tag="ss", name="ss")
                nc.scalar.activation(ss[:], r2, AF.Sqrt)
                d1 = tmp_pool.tile([P, F], bf16, tag="d1")
                _raw_scalar_act(nc, d1[:], ss[:], AF.Reciprocal, bias=EPS)
                d2 = tmp_pool.tile([P, F], bf16, tag="d2")
                _raw_scalar_act(nc, d2[:], r2, AF.Reciprocal, bias=EPS)

                vals = tmp1_pool.tile([P, 7 * F], bf16, tag="vals", name="vals")
                # dxb,dyb,dzb -> vals planes 4:7 (one CAST)
                nc.vector.tensor_copy(vals[:, 4 * F : 7 * F], dxt[:])
                # du,dv,dw -> vals planes 1:4 ; dq separate
                nc.vector.tensor_sub(
                    vals[:, 1 * F : 4 * F], cvt[:, 0 : 3 * F], evt[:, 0 : 3 * F]
                )
                dqt = tmp1_pool.tile([P, F], bf16, tag="dqt", name="dqt")
                nc.vector.tensor_sub(dqt[:], cvt[:, 3 * F :], evt[:, 3 * F :])

                # g = sum duvw*dxyz ; velgrad = g*d1 -> vals plane 0
                prt = tmp1_pool.tile([P, 3 * F], bf16, tag="prt", name="prt")
                nc.vector.tensor_mul(
                    prt[:], vals[:, 1 * F : 4 * F], vals[:, 4 * F : 7 * F]
                )
                nc.vector.tensor_add(prt[:, 0:F], prt[:, 0:F], prt[:, F : 2 * F])
                nc.vector.tensor_add(prt[:, 0:F], prt[:, 0:F], prt[:, 2 * F : 3 * F])
                nc.vector.tensor_mul(vals[:, 0:F], prt[:, 0:F], d1[:])

                # cp = dq*d1*d1 (in-place dqt)
                nc.vector.tensor_mul(dqt[:], dqt[:], d1[:])
                nc.vector.tensor_mul(dqt[:], dqt[:], d1[:])

                # pg = cp * dxyz (planes 4:7, in-place; cp broadcast over 3)
                cp3 = bass.AP(dqt[:].tensor, dqt[:].offset,
                              [dqt[:].ap[0], [0, 3], [1, F]])
                nc.vector.tensor_mul(
                    vals[:].rearrange("p (i f) -> p i f", i=7, f=F)[:, 4:7],
                    cp3,
                    vals[:].rearrange("p (i f) -> p i f", i=7, f=F)[:, 4:7],
                )
                # lap = duvw * d2 (planes 1:4, in-place; d2 broadcast over 3)
                d23 = bass.AP(d2[:].tensor, d2[:].offset,
                              [d2[:].ap[0], [0, 3], [1, F]])
                nc.vector.tensor_mul(
                    vals[:].rearrange("p (i f) -> p i f", i=7, f=F)[:, 1:4],
                    vals[:].rearrange("p (i f) -> p i f", i=7, f=F)[:, 1:4],
                    d23,
                )

                # segment sums: halving tree over K (all 7 planes at once)
                v4 = vals[:].rearrange("p (i g k) -> p i g k", i=7, g=g, k=K)
                k = K
                h = k // 2
                nc.vector.tensor_add(
                    v4[:, :, :, 0:h], v4[:, :, :, 0:h], v4[:, :, :, h:k]
                )
                k = h
                if k % 2 == 0 and (k // 2) % 2 == 0:
                    h = k // 2
                    nc.vector.tensor_add(
                        v4[:, :, :, 0:h], v4[:, :, :, 0:h], v4[:, :, :, h:k]
                    )
                    k = h
                nc.vector.tensor_reduce(
                    acc4[:, :, off_n : off_n + g],
                    v4[:, :, :, 0:k],
                    mybir.AxisListType.X,
                    OP.add,
                )

            # ---- finish (fp32, small) ----
            icnt = res_pool.tile([P, Q], fp32)
            _raw_scalar_act(nc, icnt[:], cntt[:], AF.Reciprocal)
            div = res_pool.tile([P, Q], fp32)
            nc.vector.tensor_mul(div[:], acc[:, 0:Q], icnt[:])
            acc_div2 = res_pool.tile([P, 1], fp32)
            nc.scalar.activation(div[:], div[:], AF.Square, accum_out=acc_div2[:])
            acc_m = [
                res_pool.tile([P, 1], fp32, tag=f"am{i}", name=f"am{i}")
                for i in range(3)
            ]
            for i in range(3):
                r = res_pool.tile([P, Q], fp32, tag="rfin")
                nc.vector.scalar_tensor_tensor(
                    r[:],
                    acc[:, (1 + i) * Q : (2 + i) * Q],
                    1.0 / REYNOLDS,
                    acc[:, (4 + i) * Q : (5 + i) * Q],
                    OP.mult,
                    OP.add,
                )
                nc.vector.tensor_mul(r[:], r[:], icnt[:])
                nc.scalar.activation(r[:], r[:], AF.Square, accum_out=acc_m[i][:])

            outt = res_pool.tile([P, 8], fp32)
            nc.vector.memset(outt[:], 0.0)
            nc.vector.tensor_copy(outt[:, 0:1], acc_vel[:])
            nc.vector.tensor_copy(outt[:, 1:2], acc_pres[:])
            nc.vector.tensor_copy(outt[:, 2:3], acc_div2[:])
            nc.vector.tensor_copy(outt[:, 3:4], acc_m[0][:])
            nc.vector.tensor_copy(outt[:, 4:5], acc_m[1][:])
            nc.vector.tensor_copy(outt[:, 5:6], acc_m[2][:])
            nc.sync.dma_start(out.ap()[:], outt[:])

    return nc


# ---------------------------------------------------------------- entry

_CACHE = {}


def _get_nc(key, plan, NN, S, DQ):
    if key not in _CACHE:
        _CACHE[key] = _build_nc(plan, NN, S, DQ)
    return _CACHE[key]


LAST_RESULT = None  # BassKernelResults of the most recent run (for profiling)


def kernel(pred, target, edge_index, pos, _trace_dir=None):
    global LAST_RESULT
    pred = np.asarray(pred)
    target = np.asarray(target)
    pos = np.asarray(pos)
    row = np.asarray(edge_index[0]).astype(np.int64)
    col = np.asarray(edge_index[1]).astype(np.int64)
    n = pred.shape[0]

    plan, deg, offs, order = _build_plan(row, n)
    col_sorted = col[order]
    nodedata = np.concatenate(
        [pos.astype(np.float32), pred.astype(np.float32)], axis=1
    )
    cores, NN, S = _build_streams(plan, deg, offs, col_sorted, nodedata)

    # data-loss slices: split all n nodes across cores, pad to mult of 128
    per = -(-n // N_CORES)
    DQ = (-(-per // P) * P) // P
    predf = pred.astype(np.float32)
    targf = target.astype(np.float32)

    in_maps = []
    for c in range(N_CORES):
        lo, hi = c * per, min((c + 1) * per, n)
        ps = np.zeros((P * DQ, 4), np.float32)
        ts = np.zeros((P * DQ, 4), np.float32)
        ps[: hi - lo] = predf[lo:hi]
        ts[: hi - lo] = targf[lo:hi]
        # [P, 4*DQ] with plane-major columns: plane i at cols [i*DQ, (i+1)*DQ)
        dlp = ps.reshape(P, DQ, 4).transpose(0, 2, 1).reshape(P, 4 * DQ)
        dlt = ts.reshape(P, DQ, 4).transpose(0, 2, 1).reshape(P, 4 * DQ)
        Qn = NN // P
        m = dict(
            cnt=np.ascontiguousarray(cores[c]["cnt"]),
            nodp=np.ascontiguousarray(
                cores[c]["node_pos"].transpose(1, 0, 2).reshape(P, 3 * Qn)
            ),
            nodv=np.ascontiguousarray(
                cores[c]["node_vel"].transpose(1, 0, 2).reshape(P, 4 * Qn)
            ),
            dlp=np.ascontiguousarray(dlp),
            dlt=np.ascontiguousarray(dlt),
        )
        for i in range(3):
            m[f"cpos{i}"] = np.ascontiguousarray(cores[c]["col_pos"][i])
        for i in range(4):
            m[f"cvel{i}"] = np.ascontiguousarray(cores[c]["col_vel"][i])
        in_maps.append(m)

    key = (tuple((K, m) for K, _, m in plan), NN, S, DQ)
    nc = _get_nc(key, plan, NN, S, DQ)

    if _trace_dir is not None:
        _install_ntff_hook()
        res = run_bass_kernel_spmd(
            nc, in_maps, core_ids=list(range(N_CORES)), trace=True,
            tmpdir=_trace_dir,
        )
    else:
        res = run_bass_kernel_spmd(nc, in_maps, core_ids=list(range(N_CORES)))
    LAST_RESULT = res

    tot = np.zeros(8, np.float64)
    for c in range(N_CORES):
        tot += res.results[c]["out"].astype(np.float64).sum(axis=0)
    s_vel, s_pres, s_div2, am0, am1, am2 = tot[0], tot[1], tot[2], tot[3], tot[4], tot[5]
    loss = (
        s_vel / (3 * n)
        + s_pres / n
        + LAMBDA_CONT * s_div2 / n
        + LAMBDA_MOM * (am0 + am1 + am2) / (3 * n)
    )
    return np.float32(loss)
